# revision 1
# baseline (speedup 1.0000x reference)
"""Trainium2 Bass kernel for nn_AttentionBlock_86715389706345.

Math (exact reduction of the reference):
  Per (b,h): rowsum[t] = x[t]·U[h]/sqrt(DH) with U[h] = Wq[h]@ksum[h],
  ksum = xsum@Wk[h], vsum = xsum@Wv[h] (xsum = sum_t x[t]),
  e = exp(rowsum/8 - max/8), p = e/sum(e), attn[t] = sum_h p_h[t]*vsum_h,
  z = x + attn.
  LayerNorm invariances collapse the rest:
    x1 = (z - m)*s  (s>0) =>
    relu(x1@W1) = s * relu(z@W1 - m*colsum(W1))       [relu commutes with s>0]
    LN2(x1 + relu(x1@W1)@W2) = LN2(z + relu(z@W1 - m*w1s)@W2)   [shift/scale inv]
  So only m[t] = mean(z[t]) is needed from LN1, as a rank-1 correction.
  Further split: z@W1 = x@W1 + attn@W1, and attn@W1 = e @ G with
  G[h] = (p-normalized vsum masked to head h) @ W1  (16 x 1024), so the big
  x@W1 matmul needs only x and W1 (runs early), attention joins as rank-17.

Sharding: data-parallel - batch b on core b (B == n_cores == 8), weights
replicated, no collectives.

Precision: fp32r (fp22 PE mode) everywhere; W2 stored bf16.
"""
import sys
sys.path.insert(0, '/opt/trn_rl_repo')
import numpy as np

import concourse.bass as bass
import concourse.tile as tile
import concourse.mybir as mybir
from concourse.bass_utils import run_bass_kernel_spmd
from concourse.masks import make_identity

F32 = mybir.dt.float32
BF16 = mybir.dt.bfloat16
F32R = mybir.dt.float32r
AF = mybir.ActivationFunctionType
OP = mybir.AluOpType
AX = mybir.AxisListType

B, T, D, H = 8, 1024, 1024, 16
DH = D // H
EPS = 1e-5
P = 128
NT = T // P   # 8 row tiles
NC = D // P   # 8 col tiles
N_CORES = 8
RSCALE = float(1.0 / np.sqrt(DH))


def _expand_ap(ap, reps):
    """Append a step-0 broadcast dim of size `reps` to an AP."""
    return bass.AP(tensor=ap.tensor, offset=ap.offset,
                   ap=[list(dd) for dd in ap.ap] + [[0, reps]])


def _w_ctile(wdram, j):
    """AP view of W (H, D, DH) as [c-part 128, h 16, d 64] for c-tile j."""
    base = wdram[:]
    return bass.AP(tensor=base.tensor, offset=j * P * DH,
                   ap=[[DH, P], [D * DH, H], [1, DH]])


def _split_waits(nc):
    """This container's walrus accepts ONE sync wait per instruction; Tile
    emits 2-3. Hoist extras onto single-wait NoOps on the same engine placed
    immediately before (engines execute block-order)."""
    k = 0
    for f in nc.m.functions:
        for bb in f.blocks:
            out = []
            changed = False
            for ins in bb.instructions:
                si = getattr(ins, "sync_info", None)
                if si is not None and len(si.on_wait) > 1:
                    for w in si.on_wait[:-1]:
                        nop = mybir.InstNoOp(name=f"I-waitfix-{k}")
                        k += 1
                        nop.engine = ins.engine
                        nop.sync_info = mybir.SyncInfo(on_wait=[w], on_update=[])
                        out.append(nop)
                    ins.sync_info = mybir.SyncInfo(
                        on_wait=[si.on_wait[-1]], on_update=list(si.on_update))
                    changed = True
                out.append(ins)
            if changed:
                bb.instructions = out
    return k


def build_v2(split_waits=True):
    nc = bass.Bass()
    x = nc.dram_tensor("x", [T, D], F32, kind="ExternalInput")
    Wq = nc.dram_tensor("Wq", [H, D, DH], F32, kind="ExternalInput")
    Wk = nc.dram_tensor("Wk", [H, D, DH], F32, kind="ExternalInput")
    Wv = nc.dram_tensor("Wv", [H, D, DH], F32, kind="ExternalInput")
    W1 = nc.dram_tensor("W1", [D, D], F32, kind="ExternalInput")
    W2 = nc.dram_tensor("W2", [D, D], F32, kind="ExternalInput")
    g1 = nc.dram_tensor("g1", [D], F32, kind="ExternalInput")  # noqa: F841
    b1 = nc.dram_tensor("b1", [D], F32, kind="ExternalInput")  # noqa: F841
    g2 = nc.dram_tensor("g2", [D], F32, kind="ExternalInput")  # noqa: F841
    b2 = nc.dram_tensor("b2", [D], F32, kind="ExternalInput")  # noqa: F841
    out = nc.dram_tensor("out", [T, D], F32, kind="ExternalOutput")

    xr = x.rearrange("(i p) d -> i p d", p=P)
    outr = out.rearrange("(i p) d -> i p d", p=P)
    # W1 as column slices: [b][j] -> [128 rows of c-tile j, 128 cols of f-slice b]
    w1cs = W1.rearrange("(j p) (b q) -> b j p q", p=P, q=P)
    w2r = W2.rearrange("(j p) d -> j p d", p=P)

    HALF = 512

    with tile.TileContext(nc) as tc:
        with tc.tile_pool(name="px", bufs=2) as px, \
             tc.tile_pool(name="pxT", bufs=NC) as pxT, \
             tc.tile_pool(name="pw1", bufs=1) as pw1, \
             tc.tile_pool(name="pw2", bufs=1) as pw2, \
             tc.tile_pool(name="ph1", bufs=NC) as ph1, \
             tc.tile_pool(name="pwst", bufs=8) as pwst, \
             tc.tile_pool(name="pbc", bufs=1) as pbc, \
             tc.tile_pool(name="psmall", bufs=2) as psmall, \
             tc.tile_pool(name="prow", bufs=1) as prow, \
             tc.tile_pool(name="pG", bufs=1) as pG, \
             tc.tile_pool(name="pVT", bufs=NC) as pVT, \
             tc.tile_pool(name="pconst", bufs=1) as pconst, \
             tc.tile_pool(name="ppmm", bufs=2, space="PSUM") as ppmm, \
             tc.tile_pool(name="pptr", bufs=2, space="PSUM") as pptr, \
             tc.tile_pool(name="pprow", bufs=1, space="PSUM") as pprow:

            # ---------------- constants ----------------
            identf = pconst.tile([P, P], F32)
            make_identity(nc, identf)
            ident = pconst.tile([P, P], F32R)
            nc.vector.tensor_copy(ident[:], identf[:])
            ones_f = pconst.tile([1, P], F32)
            nc.vector.memset(ones_f[:], 1.0)
            ones_row = pconst.tile([1, P], F32R)
            nc.vector.tensor_copy(ones_row[:], ones_f[:])
            ones_cf = pconst.tile([P, 1], F32)
            nc.vector.memset(ones_cf[:], 1.0)
            ones_col = pconst.tile([P, 1], F32R)
            nc.vector.tensor_copy(ones_col[:], ones_cf[:])
            eps_t = pconst.tile([P, 1], F32)
            nc.vector.memset(eps_t[:], EPS)

            # ---------------- DMA issue order (sync HWDGE ring) ----------
            # x first (gates everything), then W1 column-slices (feeds
            # mm1-main early), then Wk/Wq/Wv streamed, W2 via SWDGE bf16.
            # x alone on the sync ring: full HBM bandwidth until it lands
            xs = []
            xbigs = []
            for g in range(2):
                xb = px.tile([P, 4 * D], F32R, tag="x", bufs=2, name=f"xbig{g}")
                src_ap = bass.AP(tensor=x[:].tensor, offset=g * 4 * P * D,
                                 ap=[[D, P], [P * D, 4], [1, D]])
                nc.sync.dma_start(
                    xb[:].rearrange("p (i d) -> p i d", i=4),
                    src_ap.bitcast(F32R))
                xbigs.append(xb)
            for i in range(NT):
                xs.append(xbigs[i // 4][:, (i % 4) * D:(i % 4 + 1) * D])
            _ISSUE_ALL = True

            # weight DMAs are declared here but ISSUED from the ACT stream,
            # interleaved with compute, so data transfers self-pace instead
            # of fair-sharing HBM all at once.
            w1big = pw1.tile([P, NC * D], F32R, tag="w1")
            w1t = [w1big[:, _j * D:(_j + 1) * D] for _j in range(NC)]

            def issue_w1_slice(b, eng=None):
                dst = bass.AP(tensor=w1big[:].tensor,
                              offset=w1big[:].offset + b * P,
                              ap=[list(w1big[:].ap[0]), [D, NC], [1, P]])
                srcb = bass.AP(tensor=W1[:].tensor, offset=b * P,
                               ap=[[D, P], [P * D, NC], [1, P]])
                (eng or nc.sync).dma_start(dst, srcb.bitcast(F32R))

            wkt, wqt, wvt = [], [], []

            def issue_wst(wdram, j, lst):
                tk = pwst.tile([P, D], F32R, tag="wst",
                               name=f"wst{len(lst)}_{j}")
                nc.gpsimd.dma_start(
                    tk[:].rearrange("p (h d) -> p h d", h=H),
                    _w_ctile(wdram, j).bitcast(F32R))
                lst.append(tk)

            w1big = pw1.tile([P, NC * D], F32R, tag="w1")
            w1t = [w1big[:, _j * D:(_j + 1) * D] for _j in range(NC)]
            for b in range(NC):
                # slice b of every j-tile in one DMA: dst [p, j, q]
                dst = bass.AP(tensor=w1big[:].tensor,
                              offset=w1big[:].offset + b * P,
                              ap=[list(w1big[:].ap[0]), [D, NC], [1, P]])
                srcb = bass.AP(tensor=x[:].tensor, offset=0, ap=[[1, 1]])
                srcb = bass.AP(tensor=W1[:].tensor, offset=b * P,
                               ap=[[D, P], [P * D, NC], [1, P]])
                nc.sync.dma_start(dst, srcb.bitcast(F32R))
            # ---------------- xT transposes + xsum tree + row sums -------
            xT = [pxT.tile([P, T], F32R, tag="xT", name=f"xTt{_j}") for _j in range(NC)]
            for i in range(NT):
                for g in range(2):
                    ptr = pptr.tile([P, 512], F32R, tag="tr")
                    for u in range(4):
                        j = g * 4 + u
                        nc.tensor.transpose(
                            ptr[:, u * P:(u + 1) * P],
                            xs[i][:, j * P:(j + 1) * P].bitcast(F32R), ident[:])
                    for u in range(4):
                        j = g * 4 + u
                        dst = xT[j][:, i * P:(i + 1) * P]
                        src = ptr[:, u * P:(u + 1) * P]
                        if u % 2 == 0:
                            nc.scalar.copy(dst, src)
                        else:
                            nc.vector.tensor_copy(dst, src)
                if i == 0:
                    # Wk/Wq via SWDGE (parallel queues); W1 b0-2 on the SP
                    # ring, b3-7 paced from the ACT stream behind mm1 so the
                    # serial DMA pipe serves Wq (the critical chain) first
                    for j in range(NC):
                        issue_wst(Wk, j, wkt)
                    for j in range(NC):
                        issue_wst(Wq, j, wqt)
                    for b in range(3):
                        issue_w1_slice(b)

            # xsumT columns straight from xT (sum over t of each c-tile)
            xsumT_f = psmall.tile([P, NC], F32, tag="xsumTf")
            for j in range(NC):
                nc.vector.tensor_reduce(xsumT_f[:, j:j + 1],
                                        xT[j][:].bitcast(F32),
                                        axis=AX.X, op=OP.add)
            xsumT = psmall.tile([P, NC], F32R, tag="xsumT")
            nc.vector.tensor_copy(xsumT[:], xsumT_f[:])

            # ---------------- PE: mm1-main (interleaved with prep) -------
            h1a = []

            def mm1_block(b):
                hp = ppmm.tile([P, D], F32, tag="mm", name=f"mmh{b}")
                for j in range(NC):
                    for h2 in range(2):
                        sl = slice(h2 * HALF, (h2 + 1) * HALF)
                        nc.tensor.matmul(
                            hp[:, sl],
                            w1t[j][:, b * P:(b + 1) * P],
                            xT[j][:, sl],
                            start=(j == 0), stop=(j == NC - 1))
                hb = ph1.tile([P, D], BF16, tag="h1", name=f"h1a{b}")
                nc.scalar.copy(hb[:], hp[:])
                h1a.append(hb)
                if b < 5:
                    issue_w1_slice(b + 3, eng=nc.scalar)

            # ---------------- ksum (PE) + broadcast ----------------------
            krow_ps = pprow.tile([17, D], F32, tag="row")
            for j in range(NC):
                for h2 in range(2):
                    sl = slice(h2 * HALF, (h2 + 1) * HALF)
                    nc.tensor.matmul(krow_ps[0:1, sl],
                                     xsumT[:, j:j + 1],
                                     wkt[j][:, sl],
                                     start=(j == 0), stop=(j == NC - 1))
            ksum_row = prow.tile([1, D], F32R, tag="row")
            nc.scalar.copy(ksum_row[:], krow_ps[0:1, :])
            ksum_b = pbc.tile([P, D], F32, tag="bc")
            for h2 in range(2):
                sl = slice(h2 * HALF, (h2 + 1) * HALF)
                kb = pptr.tile([P, 512], F32, tag="tr")
                nc.tensor.matmul(kb[:], ones_row[:],
                                 ksum_row[:, sl],
                                 start=True, stop=True)
                nc.scalar.copy(ksum_b[:, sl], kb[:])

            for b in range(6):
                mm1_block(b)

            # xsum-row over d per t: ones^T @ xT, staged raw into xsr
            xmr_ps = pprow.tile([17, D], F32, tag="row")
            for j in range(NC):
                for h2 in range(2):
                    sl = slice(h2 * HALF, (h2 + 1) * HALF)
                    nc.tensor.matmul(xmr_ps[0:1, sl], ones_col[:],
                                     xT[j][:, sl],
                                     start=(j == 0), stop=(j == NC - 1))
            xsr = prow.tile([1, T], F32R, tag="xsr")
            nc.scalar.copy(xsr[:], xmr_ps[0:1, :])

            # ---------------- U = seg-reduce(Wq * ksum_b)  (DVE/Pool) ----
            UT = []
            for j in range(NC):
                eng = nc.vector if j % 2 == 0 else nc.gpsimd
                eng.tensor_mul(wqt[j][:].bitcast(F32), wqt[j][:].bitcast(F32),
                               ksum_b[:])
                utf = psmall.tile([P, H], F32, tag="UTf", bufs=1)
                nc.vector.tensor_reduce(
                    utf[:],
                    wqt[j][:].bitcast(F32).rearrange("p (h d) -> p h d", h=H),
                    axis=AX.X, op=OP.add)
                utj = psmall.tile([P, H], F32R, tag="UT", bufs=NC,
                                  name=f"UTr{j}")
                nc.scalar.copy(utj[:], utf[:])
                UT.append(utj)

            # ---------------- U = seg-reduce(Wq * ksum_b)  (DVE/Pool) ----            # ---------------- rowsum (PE) + softmax ----------------------
            rs_ps = pprow.tile([17, D], F32, tag="row")
            for j in range(NC):
                for h2 in range(2):
                    sl = slice(h2 * HALF, (h2 + 1) * HALF)
                    nc.tensor.matmul(rs_ps[0:H, sl], UT[j][:],
                                     xT[j][:, sl],
                                     start=(j == 0), stop=(j == NC - 1))
            mx = psmall.tile([H, 1], F32, tag="mx")
            nc.vector.tensor_reduce(mx[:], rs_ps[0:H, :], axis=AX.X, op=OP.max)
            negmx = psmall.tile([H, 1], F32, tag="negmx")
            nc.scalar.mul(negmx[:], mx[:], -RSCALE)
            e_ext = pG.tile([H + 1, T], F32R, tag="eext")
            sumexp = psmall.tile([H, 1], F32, tag="sumexp")
            nc.scalar.activation(e_ext[0:H, :], rs_ps[0:H, :], AF.Exp,
                                 bias=negmx[:], scale=RSCALE,
                                 accum_out=sumexp[:])
            rec = psmall.tile([H, 1], F32, tag="rec")
            nc.vector.reciprocal(rec[:], sumexp[:])
            # rec (16,1) -> rec_row (1,16)
            rr = pptr.tile([P, 512], F32, tag="tr")
            nc.tensor.transpose(rr[0:1, 0:H], rec[:], identf[0:H, 0:H])
            rec_row = prow.tile([1, H], F32, tag="rrow")
            nc.scalar.copy(rec_row[:], rr[0:1, 0:H])

            # Wv stream + W2 bf16 casts on SWDGE (Pool order: after U-muls)
            for j in range(NC):
                issue_wst(Wv, j, wvt)
            w2t = []
            for j in range(NC):
                t2 = pw2.tile([P, D], BF16, tag="w2", bufs=NC, name=f"w2c{j}")
                nc.gpsimd.dma_start(t2[:], w2r[j])
                w2t.append(t2)

            mm1_block(6)
            mm1_block(7)

            # ---------------- vsum (PE) + vsum_n -------------------------
            vrow_ps = pprow.tile([17, D], F32, tag="row")
            for j in range(NC):
                for h2 in range(2):
                    sl = slice(h2 * HALF, (h2 + 1) * HALF)
                    nc.tensor.matmul(vrow_ps[0:1, sl],
                                     xsumT[:, j:j + 1],
                                     wvt[j][:, sl],
                                     start=(j == 0), stop=(j == NC - 1))
            vsum_nf = prow.tile([1, D], F32R, tag="row")
            nc.vector.tensor_tensor(
                out=vsum_nf[:].rearrange("p (h d) -> p h d", h=H),
                in0=vrow_ps[0:1, :].rearrange("p (h d) -> p h d", h=H),
                in1=_expand_ap(rec_row[:], DH), op=OP.mult)
            # vs_m_raw[h] = sum_d vsum_n[h, d]  (as [1,16] row)
            vs_m_row = prow.tile([1, H], F32, tag="vsm")
            nc.vector.tensor_reduce(
                vs_m_row[:],
                vsum_nf[:].bitcast(F32).rearrange("p (h d) -> p h d", h=H),
                axis=AX.X, op=OP.add)

            # vs_m as column [16,1]: f32-mode transpose, then rounding copy
            vtr2 = pptr.tile([P, 512], F32, tag="tr")
            nc.tensor.transpose(vtr2[0:H, 0:1], vs_m_row[:], identf[0:1, 0:1])
            vsm_col = psmall.tile([H, 1], F32R, tag="vsmc")
            nc.scalar.copy(vsm_col[:], vtr2[0:H, 0:1])

            # Vblk [16, 1024]: broadcast vsum_n then mask to block-diagonal
            vb_ps = pprow.tile([17, D], F32, tag="row")
            for h2 in range(2):
                sl = slice(h2 * HALF, (h2 + 1) * HALF)
                nc.tensor.matmul(vb_ps[0:H, sl], ones_row[:, 0:H],
                                 vsum_nf[:, sl],
                                 start=True, stop=True)
            vblk_f = prow.tile([H, D], F32, tag="row")
            nc.scalar.copy(vblk_f[:], vb_ps[0:H, :])
            # block-diagonal mask = identity16 broadcast over d
            nc.gpsimd.tensor_tensor(
                out=vblk_f[:].rearrange("p (h d) -> p h d", h=H),
                in0=vblk_f[:].rearrange("p (h d) -> p h d", h=H),
                in1=_expand_ap(identf[0:H, 0:H], DH), op=OP.mult)
            Vblk = pG.tile([H, D], F32R, tag="vblk")
            nc.scalar.copy(Vblk[:], vblk_f[:])

            # VTx[j] = transpose(vblk_f block j) [128, 16] (+ ones col 16)
            VTx = []
            for j in range(NC):
                vtp = pptr.tile([P, 512], F32, tag="tr", name=f"vtp{j}")
                nc.tensor.transpose(vtp[:, 0:H],
                                    vblk_f[:, j * P:(j + 1) * P],
                                    identf[0:H, 0:H])
                vt = pVT.tile([P, H + 1], F32R, tag="VT", name=f"VT{j}")
                nc.vector.tensor_copy(vt[:, 0:H], vtp[:, 0:H])
                nc.vector.tensor_copy(vt[:, H:H + 1], ones_col[:])
                VTx.append(vt)

            # ---------------- G3 = [VT|ones]^T @ W1  ([17, D]) -----------
            g_ps = pprow.tile([17, D], F32, tag="row")
            for j in range(NC):
                for h2 in range(2):
                    sl = slice(h2 * HALF, (h2 + 1) * HALF)
                    nc.tensor.matmul(g_ps[:, sl], VTx[j][:],
                                     w1t[j][:, sl],
                                     start=(j == 0), stop=(j == NC - 1))
            G3 = pG.tile([H + 1, D], F32R, tag="G3")
            nc.scalar.copy(G3[:], g_ps[:])

            # ---------------- e_ext row 16 = -(xsum_row + e@vs_m)/D ------
            nx_ps = pprow.tile([17, D], F32, tag="row")
            for h2 in range(2):
                sl = slice(h2 * HALF, (h2 + 1) * HALF)
                nc.tensor.matmul(nx_ps[0:1, sl], vsm_col[:],
                                 e_ext[0:H, sl],
                                 start=True, stop=False)
                nc.tensor.matmul(nx_ps[0:1, sl], ones_row[:, 0:1],
                                 xsr[:, sl],
                                 start=False, stop=True)
            negm_row = prow.tile([1, T], F32R, tag="xsr")
            nc.scalar.mul(negm_row[:], nx_ps[0:1, :], -1.0 / D)
            nc.scalar.dma_start(e_ext[H:H + 1, :], negm_row[:])

            # ---------------- pass2: h1s = relu(h1a + e_ext @ G3) --------
            h1s = []
            for b in range(NC):
                atp = ppmm.tile([P, D], F32, tag="mm")
                for h2 in range(2):
                    sl = slice(h2 * HALF, (h2 + 1) * HALF)
                    nc.tensor.matmul(atp[:, sl],
                                     G3[:, b * P:(b + 1) * P],
                                     e_ext[:, sl],
                                     start=True, stop=True)
                hs = pxT.tile([P, T], BF16, tag="xT")
                nc.vector.tensor_add(hs[:], h1a[b][:], atp[:])
                nc.scalar.activation(hs[:], hs[:], AF.Relu)
                h1s.append(hs)

            # ---------------- mm2 + residuals + LN2 (staged pipeline) ----
            BNS = nc.vector.BN_STATS_DIM
            BNA = nc.vector.BN_AGGR_DIM
            z3s, mvs, ss = [], [], []
            for i in range(NT):
                zp = ppmm.tile([P, D], F32, tag="mm")
                # attention term: e-block^T @ Vblk
                for h2 in range(2):
                    sl = slice(h2 * HALF, (h2 + 1) * HALF)
                    nc.tensor.matmul(zp[:, sl],
                                     e_ext[0:H, i * P:(i + 1) * P],
                                     Vblk[:, sl],
                                     start=True, stop=False)
                for f in range(NC):
                    for h2 in range(2):
                        sl = slice(h2 * HALF, (h2 + 1) * HALF)
                        nc.tensor.matmul(zp[:, sl],
                                         h1s[f][:, i * P:(i + 1) * P],
                                         w2t[f][:, sl],
                                         start=False, stop=(f == NC - 1))
                z3 = ph1.tile([P, D], F32, tag="h1", bufs=NT, name=f"z3_{i}")
                nc.vector.tensor_add(z3[:], xs[i].bitcast(F32), zp[:])
                stats = psmall.tile([P, 2, BNS], F32, tag="stats")
                zr = z3[:].rearrange("p (g d) -> p g d", g=2)
                nc.vector.bn_stats(out=stats[:, 0, :], in_=zr[:, 0, :])
                nc.vector.bn_stats(out=stats[:, 1, :], in_=zr[:, 1, :])
                mv = psmall.tile([P, BNA], F32, tag="mv", bufs=NT, name=f"mv{i}")
                nc.vector.bn_aggr(out=mv[:], in_=stats[:])
                s = psmall.tile([P, 1], F32, tag="s", bufs=NT, name=f"s{i}")
                nc.scalar.activation(s[:], mv[:, 1:2], AF.Sqrt, bias=eps_t[:])
                z3s.append(z3); mvs.append(mv); ss.append(s)
                # flush finished half-batch: its recip/apply/store overlap
                # the remaining mm2 tiles instead of trailing the last one
                if i in (3, NT - 1):
                    lo = 0 if i == 3 else 4
                    for k in range(lo, i + 1):
                        nc.vector.reciprocal(ss[k][:], ss[k][:])
                    for k in range(lo, i + 1):
                        nc.vector.tensor_scalar(
                            out=z3s[k][:], in0=z3s[k][:],
                            scalar1=mvs[k][:, 0:1], scalar2=ss[k][:],
                            op0=OP.subtract, op1=OP.mult)
                        nc.scalar.dma_start(outr[k], z3s[k][:])

    if split_waits:
        _split_waits(nc)
    return nc


# ---------------------------------------------------------------------------
# v1 baseline build (used for non-trivial gamma/beta inputs)
# ---------------------------------------------------------------------------

def build_v1(trivial_gb=False):
    nc = bass.Bass()
    x = nc.dram_tensor("x", [T, D], F32, kind="ExternalInput")
    Wq = nc.dram_tensor("Wq", [H, D, DH], F32, kind="ExternalInput")
    Wk = nc.dram_tensor("Wk", [H, D, DH], F32, kind="ExternalInput")
    Wv = nc.dram_tensor("Wv", [H, D, DH], F32, kind="ExternalInput")
    W1 = nc.dram_tensor("W1", [D, D], F32, kind="ExternalInput")
    W2 = nc.dram_tensor("W2", [D, D], F32, kind="ExternalInput")
    g1 = nc.dram_tensor("g1", [D], F32, kind="ExternalInput")
    b1 = nc.dram_tensor("b1", [D], F32, kind="ExternalInput")
    g2 = nc.dram_tensor("g2", [D], F32, kind="ExternalInput")
    b2 = nc.dram_tensor("b2", [D], F32, kind="ExternalInput")
    out = nc.dram_tensor("out", [T, D], F32, kind="ExternalOutput")

    xr = x.rearrange("(i p) d -> i p d", p=P)
    outr = out.rearrange("(i p) d -> i p d", p=P)
    w1r = W1.rearrange("(j p) d -> j p d", p=P)
    w2r = W2.rearrange("(j p) d -> j p d", p=P)

    with tile.TileContext(nc) as tc:
        with tc.tile_pool(name="px", bufs=2) as px, \
             tc.tile_pool(name="pxT", bufs=NC) as pxT, \
             tc.tile_pool(name="pw1", bufs=1) as pw1, \
             tc.tile_pool(name="pw2", bufs=1) as pw2, \
             tc.tile_pool(name="px1T", bufs=NC) as px1T, \
             tc.tile_pool(name="pwst", bufs=2) as pwst, \
             tc.tile_pool(name="pbc", bufs=5) as pbc, \
             tc.tile_pool(name="ptmp", bufs=3) as ptmp, \
             tc.tile_pool(name="pz2", bufs=2) as pz2, \
             tc.tile_pool(name="psmall", bufs=8) as psmall, \
             tc.tile_pool(name="prow", bufs=1) as prow, \
             tc.tile_pool(name="pconst", bufs=1) as pconst, \
             tc.tile_pool(name="ppt", bufs=2, space="PSUM") as ppt, \
             tc.tile_pool(name="ppm", bufs=2, space="PSUM") as ppm, \
             tc.tile_pool(name="ppa", bufs=1, space="PSUM") as ppa:

            ident = pconst.tile([P, P], F32)
            make_identity(nc, ident)
            ones_f = pconst.tile([1, P], F32)
            nc.vector.memset(ones_f[:], 1.0)
            ones_col = pconst.tile([1, P], F32R)
            nc.vector.tensor_copy(ones_col[:], ones_f[:])
            eps_t = pconst.tile([P, 1], F32)
            nc.vector.memset(eps_t[:], EPS)

            xs = []
            xbigs = []
            for g in range(2):
                xb = px.tile([P, 4 * D], F32R, tag="x", bufs=2, name=f"xbig{g}")
                src_ap = bass.AP(tensor=x[:].tensor, offset=g * 4 * P * D,
                                 ap=[[D, P], [P * D, 4], [1, D]])
                nc.sync.dma_start(
                    xb[:].rearrange("p (i d) -> p i d", i=4),
                    src_ap.bitcast(F32R))
                xbigs.append(xb)
            for i in range(NT):
                xs.append(xbigs[i // 4][:, (i % 4) * D:(i % 4 + 1) * D])
            _ISSUE_ALL = True

            w1b, w2b = [], []
            for j in range(NC):
                t1 = pw1.tile([P, D], BF16, tag="w1")
                nc.gpsimd.dma_start(t1[:], w1r[j])
                w1b.append(t1)
            for j in range(NC):
                t2 = pw2.tile([P, D], BF16, tag="w2")
                nc.gpsimd.dma_start(t2[:], w2r[j])
                w2b.append(t2)

            def bcast_vec(v):
                tt = pbc.tile([P, D], F32, tag="bc")
                srcap = bass.AP(tensor=v[:].tensor, offset=0, ap=[[0, P], [1, D]])
                nc.gpsimd.dma_start(tt[:], srcap)
                return tt
            if not trivial_gb:
                g1bc = bcast_vec(g1)
                b1bc = bcast_vec(b1)
                g2bc = bcast_vec(g2)
                b2bc = bcast_vec(b2)

            a01 = ptmp.tile([P, D], F32, tag="acc")
            nc.vector.tensor_add(a01[:], xs[0][:], xs[1][:])
            a23 = ptmp.tile([P, D], F32, tag="acc")
            nc.gpsimd.tensor_add(a23[:], xs[2][:], xs[3][:])
            a03 = ptmp.tile([P, D], F32, tag="acc")
            nc.vector.tensor_add(a03[:], a01[:], a23[:])
            a45 = ptmp.tile([P, D], F32, tag="acc")
            nc.vector.tensor_add(a45[:], xs[4][:], xs[5][:])
            a67 = ptmp.tile([P, D], F32, tag="acc")
            nc.gpsimd.tensor_add(a67[:], xs[6][:], xs[7][:])
            nc.vector.tensor_add(a45[:], a45[:], a67[:])
            nc.vector.tensor_add(a45[:], a03[:], a45[:])
            acc = a45

            xsumT_f = psmall.tile([P, NC], F32, tag="xsumTf")
            for g in range(2):
                ptr = ppt.tile([P, 512], F32, tag="tr")
                for u in range(4):
                    j = g * 4 + u
                    nc.tensor.transpose(ptr[:, u * P:(u + 1) * P],
                                        acc[:, j * P:(j + 1) * P], ident[:])
                nc.vector.tensor_reduce(
                    xsumT_f[:, g * 4:(g + 1) * 4],
                    ptr[:].rearrange("p (u q) -> p u q", u=4),
                    axis=AX.X, op=OP.add)
            xsumT = psmall.tile([P, NC], F32R, tag="xsumT")
            nc.vector.tensor_copy(xsumT[:], xsumT_f[:])

            xT = []
            for j in range(NC):
                tj = pxT.tile([P, T], F32R, tag="xT")
                for g in range(2):
                    ptr = ppt.tile([P, 512], F32, tag="tr")
                    for u in range(4):
                        i = g * 4 + u
                        nc.tensor.transpose(ptr[:, u * P:(u + 1) * P],
                                            xs[i][:, j * P:(j + 1) * P], ident[:])
                    nc.vector.tensor_copy(tj[:, g * 512:(g + 1) * 512], ptr[:])
                xT.append(tj)

            def sum_row(wdram, name):
                ps_row = ppa.tile([H, T], F32, tag="arow")
                for j in range(NC):
                    wj = pwst.tile([P, D], F32R, tag="wst")
                    nc.sync.dma_start(
                        wj[:].rearrange("p (h d) -> p h d", h=H),
                        _w_ctile(wdram, j).bitcast(F32R))
                    lhs = xsumT[:, j:j + 1]
                    for h2 in range(2):
                        sl = slice(h2 * 512, (h2 + 1) * 512)
                        nc.tensor.matmul(ps_row[0:1, sl], lhs, wj[:, sl],
                                         start=(j == 0), stop=(j == NC - 1))
                row = prow.tile([1, D], F32R, tag=name)
                nc.vector.tensor_copy(row[:], ps_row[0:1, :])
                return row
            ksum_row = sum_row(Wk, "krow")

            kb_ps = ppm.tile([P, D], F32, tag="mm")
            for h2 in range(2):
                sl = slice(h2 * 512, (h2 + 1) * 512)
                nc.tensor.matmul(kb_ps[:, sl], ones_col[:], ksum_row[:, sl],
                                 start=True, stop=True)
            ksum_b = pbc.tile([P, D], F32, tag="bc")
            nc.vector.tensor_copy(ksum_b[:], kb_ps[:])

            vsum_row = sum_row(Wv, "vrow")

            UT = []
            for j in range(NC):
                wqj = pwst.tile([P, D], F32, tag="wst")
                nc.gpsimd.dma_start(
                    wqj[:].rearrange("p (h d) -> p h d", h=H), _w_ctile(Wq, j))
                prod = ptmp.tile([P, D], F32, tag="acc")
                nc.vector.tensor_mul(prod[:], wqj[:], ksum_b[:])
                utf = psmall.tile([P, H], F32, tag="UTf")
                nc.vector.tensor_reduce(
                    utf[:], prod[:].rearrange("p (h d) -> p h d", h=H),
                    axis=AX.X, op=OP.add)
                utj = psmall.tile([P, H], F32R, tag="UT")
                nc.vector.tensor_copy(utj[:], utf[:])
                UT.append(utj)

            rs_ps = ppa.tile([H, T], F32, tag="arow")
            for j in range(NC):
                for h2 in range(2):
                    sl = slice(h2 * 512, (h2 + 1) * 512)
                    nc.tensor.matmul(rs_ps[:, sl], UT[j][:], xT[j][:, sl],
                                     start=(j == 0), stop=(j == NC - 1))

            mx = psmall.tile([H, 1], F32, tag="mx")
            nc.vector.tensor_reduce(mx[:], rs_ps[:], axis=AX.X, op=OP.max)
            negmx = psmall.tile([H, 1], F32, tag="negmx")
            nc.scalar.mul(negmx[:], mx[:], -RSCALE)
            e_sb = prow.tile([H, T], F32, tag="esb")
            sumexp = psmall.tile([H, 1], F32, tag="sumexp")
            nc.scalar.activation(e_sb[:], rs_ps[:], AF.Exp,
                                 bias=negmx[:], scale=RSCALE,
                                 accum_out=sumexp[:])
            rec = psmall.tile([H, 1], F32, tag="rec")
            nc.vector.reciprocal(rec[:], sumexp[:])

            rr_ps = ppt.tile([P, 512], F32, tag="tr")
            nc.tensor.transpose(rr_ps[0:1, 0:H], rec[:], ident[:H, :H])
            rec_row = prow.tile([1, H], F32, tag="recrow")
            nc.scalar.copy(rec_row[:], rr_ps[0:1, 0:H])

            vsum_nf = prow.tile([1, D], F32R, tag="row")
            nc.vector.tensor_tensor(
                out=vsum_nf[:].rearrange("p (h d) -> p h d", h=H),
                in0=vsum_row[:].bitcast(F32).rearrange("p (h d) -> p h d", h=H),
                in1=_expand_ap(rec_row[:], DH), op=OP.mult)
            vsum_n = prow.tile([1, D], F32R, tag="vn")
            nc.vector.tensor_copy(vsum_n[:], vsum_nf[:])

            vb_ps = ppm.tile([P, D], F32, tag="mm")
            for h2 in range(2):
                sl = slice(h2 * 512, (h2 + 1) * 512)
                nc.tensor.matmul(vb_ps[:, sl], ones_col[:], vsum_n[:, sl],
                                 start=True, stop=True)
            vsum_b = pbc.tile([P, D], F32, tag="bc")
            nc.vector.tensor_copy(vsum_b[:], vb_ps[:])

            eT = []
            for i in range(NT):
                et_ps = ppt.tile([P, 512], F32, tag="tr")
                nc.tensor.transpose(et_ps[:, 0:H],
                                    e_sb[:, i * P:(i + 1) * P], ident[:H, :H])
                ei = psmall.tile([P, H], F32, tag="eT")
                nc.scalar.copy(ei[:], et_ps[:, 0:H])
                eT.append(ei)

            BNS = nc.vector.BN_STATS_DIM
            BNA = nc.vector.BN_AGGR_DIM
            for i in range(NT):
                xi = xs[i]
                tmp = ptmp.tile([P, D], F32, tag="acc")
                nc.vector.tensor_tensor(
                    out=tmp[:].rearrange("p (h d) -> p h d", h=H),
                    in0=vsum_b[:].rearrange("p (h d) -> p h d", h=H),
                    in1=_expand_ap(eT[i][:], DH), op=OP.mult)
                nc.vector.tensor_add(xi[:], xi[:], tmp[:])
                stats = psmall.tile([P, 2, BNS], F32, tag="stats")
                zr = xi[:].rearrange("p (g d) -> p g d", g=2)
                nc.vector.bn_stats(out=stats[:, 0, :], in_=zr[:, 0, :])
                nc.vector.bn_stats(out=stats[:, 1, :], in_=zr[:, 1, :])
                mv = psmall.tile([P, BNA], F32, tag="mv")
                nc.vector.bn_aggr(out=mv[:], in_=stats[:])
                s = psmall.tile([P, 1], F32, tag="s")
                nc.scalar.activation(s[:], mv[:, 1:2], AF.Sqrt, bias=eps_t[:])
                nc.vector.reciprocal(s[:], s[:])
                if trivial_gb:
                    nc.vector.tensor_scalar(
                        out=xi[:], in0=xi[:], scalar1=mv[:, 0:1], scalar2=s[:],
                        op0=OP.subtract, op1=OP.mult)
                else:
                    ms = psmall.tile([P, 1], F32, tag="ms")
                    nc.vector.tensor_mul(ms[:], mv[:, 0:1], s[:])
                    nc.scalar.activation(xi[:], xi[:], AF.Copy, bias=0.0,
                                         scale=s[:])
                    nc.vector.scalar_tensor_tensor(
                        out=xi[:], in0=xi[:], scalar=ms[:], in1=g1bc[:],
                        op0=OP.subtract, op1=OP.mult)
                    nc.gpsimd.tensor_add(xi[:], xi[:], b1bc[:])

            x1T = []
            for _j in range(NC):
                tjx = px1T.tile([P, T], BF16, tag="x1T")
                x1T.append(tjx)
            for i in range(NT):
                for g in range(2):
                    ptr = ppt.tile([P, 512], F32, tag="tr")
                    for u in range(4):
                        j = g * 4 + u
                        nc.tensor.transpose(ptr[:, u * P:(u + 1) * P],
                                            xs[i][:, j * P:(j + 1) * P],
                                            ident[:])
                    for u in range(4):
                        j = g * 4 + u
                        dst = x1T[j][:, i * P:(i + 1) * P]
                        if j % 2 == 0:
                            nc.scalar.copy(dst, ptr[:, u * P:(u + 1) * P])
                        else:
                            nc.vector.tensor_copy(dst, ptr[:, u * P:(u + 1) * P])

            h1T = []
            for f in range(NC):
                hp = ppm.tile([P, T], F32, tag="mm")
                for c in range(NC):
                    for h2 in range(2):
                        sl = slice(h2 * 512, (h2 + 1) * 512)
                        nc.tensor.matmul(hp[:, sl],
                                         w1b[c][:, f * P:(f + 1) * P],
                                         x1T[c][:, sl],
                                         start=(c == 0), stop=(c == NC - 1))
                hf = pxT.tile([P, T], BF16, tag="xT")
                nc.scalar.activation(hf[:], hp[:], AF.Relu)
                h1T.append(hf)

            for i in range(NT):
                fp = ppm.tile([P, D], F32, tag="mm")
                for f in range(NC):
                    for h2 in range(2):
                        sl = slice(h2 * 512, (h2 + 1) * 512)
                        nc.tensor.matmul(fp[:, sl],
                                         h1T[f][:, i * P:(i + 1) * P],
                                         w2b[f][:, sl],
                                         start=(f == 0), stop=(f == NC - 1))
                z2 = pz2.tile([P, D], F32, tag="z2")
                nc.vector.tensor_add(z2[:], fp[:], xs[i][:])
                stats2 = psmall.tile([P, 2, BNS], F32, tag="stats")
                z2r = z2[:].rearrange("p (g d) -> p g d", g=2)
                nc.vector.bn_stats(out=stats2[:, 0, :], in_=z2r[:, 0, :])
                nc.vector.bn_stats(out=stats2[:, 1, :], in_=z2r[:, 1, :])
                mv2 = psmall.tile([P, BNA], F32, tag="mv")
                nc.vector.bn_aggr(out=mv2[:], in_=stats2[:])
                s2 = psmall.tile([P, 1], F32, tag="s")
                nc.scalar.activation(s2[:], mv2[:, 1:2], AF.Sqrt, bias=eps_t[:])
                nc.vector.reciprocal(s2[:], s2[:])
                if trivial_gb:
                    nc.vector.tensor_scalar(
                        out=z2[:], in0=z2[:], scalar1=mv2[:, 0:1], scalar2=s2[:],
                        op0=OP.subtract, op1=OP.mult)
                else:
                    ms2 = psmall.tile([P, 1], F32, tag="ms")
                    nc.vector.tensor_mul(ms2[:], mv2[:, 0:1], s2[:])
                    nc.scalar.activation(z2[:], z2[:], AF.Copy, bias=0.0,
                                         scale=s2[:])
                    nc.vector.scalar_tensor_tensor(
                        out=z2[:], in0=z2[:], scalar=ms2[:], in1=g2bc[:],
                        op0=OP.subtract, op1=OP.mult)
                    nc.gpsimd.tensor_add(z2[:], z2[:], b2bc[:])
                nc.scalar.dma_start(outr[i], z2[:])

    _split_waits(nc)
    return nc


_NC_CACHE = {}
PROFILE = False          # set True by test harness to capture an NTFF trace
LAST_RESULT = None       # BassKernelResults of the last call (for exec_time_ns)


def kernel(x, Wq, Wk, Wv, W1, W2, g1, b1, g2, b2):
    trivial = (np.all(np.asarray(g1) == 1.0) and np.all(np.asarray(b1) == 0.0)
               and np.all(np.asarray(g2) == 1.0) and np.all(np.asarray(b2) == 0.0))
    key = "v2" if trivial else "v1"
    if key not in _NC_CACHE:
        _NC_CACHE[key] = build_v2() if trivial else build_v1(trivial_gb=False)
    nc = _NC_CACHE[key]
    common = {"Wq": np.ascontiguousarray(Wq, np.float32),
              "Wk": np.ascontiguousarray(Wk, np.float32),
              "Wv": np.ascontiguousarray(Wv, np.float32),
              "W1": np.ascontiguousarray(W1, np.float32),
              "W2": np.ascontiguousarray(W2, np.float32),
              "g1": np.ascontiguousarray(g1, np.float32),
              "b1": np.ascontiguousarray(b1, np.float32),
              "g2": np.ascontiguousarray(g2, np.float32),
              "b2": np.ascontiguousarray(b2, np.float32)}
    in_maps = [dict(common, x=np.ascontiguousarray(np.asarray(x)[b], np.float32))
               for b in range(B)]
    global LAST_RESULT
    res = run_bass_kernel_spmd(nc, in_maps, list(range(N_CORES)),
                               **({"trace": True} if PROFILE else {}))
    LAST_RESULT = res
    return np.stack([res.results[b]["out"] for b in range(B)], axis=0)

